# revision 1
# baseline (speedup 1.0000x reference)
"""BinaryConv2d Trainium2 kernel (8-core batch-parallel, PE row-group packed).

Per image: top half rows on partitions 0:64, bottom half on 64:128 (each a
zero-padded bf16 slab of HALF+2 rows). Each 3x3 conv position runs as TWO
concurrent 64x64 matmuls on PE row groups (tile_position (0,0) and (64,0)),
accumulating into separate PSUM banks. Doubles PE utilization vs v1.
"""
import sys
import numpy as np
from contextlib import ExitStack

sys.path.insert(0, "/root/.axon_site/_ro/trn_rl_repo")
sys.path.insert(0, "/opt/trn_rl_repo")

import ml_dtypes
import concourse.bass as bass
import concourse.bacc as bacc
import concourse.mybir as mybir
import concourse.tile as tile
from concourse.bass_utils import run_bass_kernel_spmd

F32 = mybir.dt.float32
BF16 = mybir.dt.bfloat16

N_CORES = 8
B, CIN, COUT, KS = 32, 64, 64, 3
H = W = 160
B_CORE = B // N_CORES
HALF = H // 2          # rows per half
SH = HALF + 2          # slab rows per half (1 halo/pad row each side)
PW = W + 2
RPT = 3                # output rows per PSUM tile


def build_nc(n_img=B_CORE, h=H, w=W):
    half = h // 2
    sh = half + 2
    pw = w + 2
    nc = bacc.Bacc("TRN2", target_bir_lowering=False, debug=False, num_devices=N_CORES)
    x_in = nc.declare_dram_parameter("x", [n_img, CIN, h, w], F32, isOutput=False)
    wsgn_in = nc.declare_dram_parameter("wsgn", [128, 9 * 64], BF16, isOutput=False)
    scale_in = nc.declare_dram_parameter("scale", [64, 1], F32, isOutput=False)
    out_ext = nc.declare_dram_parameter("out", [n_img, COUT, h, w], F32, isOutput=True)

    n_tiles = (half + RPT - 1) // RPT

    with tile.TileContext(nc) as tc, ExitStack() as ctx:
        wpool = ctx.enter_context(tc.tile_pool(name="wpool", bufs=1))
        spool = ctx.enter_context(tc.tile_pool(name="spool", bufs=1))
        xpool = ctx.enter_context(tc.tile_pool(name="xpool", bufs=3))
        ppool = ctx.enter_context(tc.tile_pool(name="ppool", bufs=3, space="PSUM"))
        opool = ctx.enter_context(tc.tile_pool(name="opool", bufs=8))

        wt2 = wpool.tile([128, 9 * 64], BF16, name="wt2")
        nc.sync.dma_start(wt2[:], wsgn_in[:])
        sc = wpool.tile([64, 1], F32, name="sc")
        nc.sync.dma_start(sc[:], scale_in[:])

        # Two persistent slab buffers (manual ping-pong); pads zeroed once.
        slabs = []
        for i in range(2):
            slab = spool.tile([128, sh * pw], BF16, name=f"slab{i}", tag=f"slab{i}")
            s3 = slab.rearrange("p (r c) -> p r c", c=pw)
            # col pads: elements r*pw + {0, pw-1} for all slab rows
            nc.vector.memset(slab[:, 0 : (sh - 1) * pw + pw : pw], 0.0)
            nc.vector.memset(slab[:, pw - 1 : sh * pw : pw], 0.0)
            # row pads: top half row 0 (partitions 0:64), bottom half last row
            nc.vector.memset(s3[0:64, 0, :], 0.0)
            nc.vector.memset(s3[64:128, sh - 1, :], 0.0)
            slabs.append(slab)

        # staging: slab rows 1..half in chunks; leftovers done separately.
        # Finer chunks shorten the critical path to the image's first matmul.
        n_ch = 4 if half % 4 == 0 else (2 if half % 2 == 0 else 1)
        ch = half // n_ch  # slab rows per chunk (covers s=1..half)

        for img in range(n_img):
            slab = slabs[img % 2]
            s3 = slab.rearrange("p (r c) -> p r c", c=pw)

            # halo rows FIRST: bottom slab row 0 <- x row half-1 gates the very
            # first bottom-half tile, so it must not queue behind the big signs
            xs = xpool.tile([128, w], F32, name="xs", tag="xs")
            nc.sync.dma_start(xs[0:64, :], x_in[img, :, half : half + 1, :])
            nc.sync.dma_start(xs[64:128, :], x_in[img, :, half - 1 : half, :])
            nc.scalar.sign(s3[0:64, sh - 1, 1 : 1 + w], xs[0:64, :])
            nc.scalar.sign(s3[64:128, 0, 1 : 1 + w], xs[64:128, :])

            for c in range(n_ch):
                s_lo = 1 + c * ch  # slab row range [s_lo, s_lo+ch)
                xc = xpool.tile([128, ch * w], F32, name="xc", tag="xc")
                xc3 = xc.rearrange("p (r c) -> p r c", c=w)
                # top half: slab row s <- x row s-1
                nc.sync.dma_start(
                    xc[0:64, :], x_in[img, :, s_lo - 1 : s_lo - 1 + ch, :]
                )
                # bottom half: slab row s <- x row half - 1 + s
                nc.sync.dma_start(
                    xc[64:128, :],
                    x_in[img, :, half - 1 + s_lo : half - 1 + s_lo + ch, :],
                )
                nc.scalar.sign(s3[:, s_lo : s_lo + ch, 1 : 1 + w], xc3[:])

            for t in range(n_tiles):
                h0 = t * RPT
                R = min(RPT, half - h0)
                psumT = ppool.tile([64, R * w], F32, name="psumT", tag="psumT")
                psumB = ppool.tile([64, R * w], F32, name="psumB", tag="psumB")
                for kh in range(KS):
                    for kw in range(KS):
                        pos = kh * KS + kw
                        st, sp = (pos == 0), (pos == 8)
                        nc.tensor.matmul(
                            psumT[:],
                            wt2[0:64, pos * 64 : (pos + 1) * 64],
                            s3[0:64, h0 + kh : h0 + kh + R, kw : kw + w],
                            start=st, stop=sp,
                            tile_position=(0, 0),
                        )
                        nc.tensor.matmul(
                            psumB[:],
                            wt2[64:128, pos * 64 : (pos + 1) * 64],
                            s3[64:128, h0 + kh : h0 + kh + R, kw : kw + w],
                            start=st, stop=sp,
                            tile_position=(64, 0),
                        )
                # DVE drains both (ACT is ~2.6x slower at PSUM copies and is
                # kept free for the binarize sign ops)
                otT = opool.tile([64, R * w], F32, name="otT", tag="otT")
                nc.vector.tensor_scalar_mul(otT[:], psumT[:], sc[:])
                nc.sync.dma_start(out_ext[img, :, h0 : h0 + R, :], otT[:])
                otB = opool.tile([64, R * w], F32, name="otB", tag="otB")
                nc.vector.tensor_scalar_mul(otB[:], psumB[:], sc[:])
                nc.sync.dma_start(
                    out_ext[img, :, half + h0 : half + h0 + R, :], otB[:]
                )
    nc.finalize()
    return nc


_NC_CACHE = {}


def _get_nc():
    if "nc" not in _NC_CACHE:
        _NC_CACHE["nc"] = build_nc()
    return _NC_CACHE["nc"]


def _prep_weights(w):
    wc = np.clip(np.asarray(w, dtype=np.float32), -1.0, 1.0)
    scale = np.abs(wc).mean(axis=(1, 2, 3)).astype(np.float32).reshape(64, 1)
    s = np.sign(wc).astype(np.float32)  # [co, ci, kh, kw]
    wsgn = np.ascontiguousarray(
        np.transpose(s, (1, 2, 3, 0)).reshape(64, 9 * 64)
    )
    wsgn2 = np.concatenate([wsgn, wsgn], axis=0).astype(ml_dtypes.bfloat16)
    return wsgn2, scale


def kernel(x, w, _trace=False):
    x = np.ascontiguousarray(np.asarray(x, dtype=np.float32))
    wsgn2, scale = _prep_weights(w)
    nc = _get_nc()
    in_maps = [
        {"x": x[i * B_CORE : (i + 1) * B_CORE], "wsgn": wsgn2, "scale": scale}
        for i in range(N_CORES)
    ]
    # The axon-proxied execution occasionally faults with a transient
    # NRT_EXEC_UNIT_UNRECOVERABLE; a retry on a fresh session recovers.
    last_err = None
    for attempt in range(3):
        try:
            res = run_bass_kernel_spmd(nc, in_maps, list(range(N_CORES)), trace=_trace)
            break
        except Exception as e:  # noqa: BLE001
            last_err = e
            import time as _time
            _time.sleep(3.0)
    else:
        raise last_err
    out = np.concatenate([res.results[i]["out"] for i in range(N_CORES)], axis=0)
    if _trace:
        return out, res
    return out



# revision 2
# speedup vs baseline: 1.1184x; 1.1184x over previous
"""BinaryConv2d Trainium2 kernel (8-core batch-parallel, full 2x2 PE quadrant
packing).

Per image the 160 output rows split into 4 quarters of 40 rows; each quarter
is one 64x64 PE quadrant (tile_position (0,0)/(64,0)/(0,64)/(64,64)), so all
128x128 PEs are busy: quarters 0/2 stream from SBUF partitions 0:63/64:127 of
slabA, quarters 1/3 from slabB.  Each 3x3 position is 4 concurrent 64x64
matmuls accumulating into two PSUM tiles (bankA = quarters 0+1 on partitions
0:63/64:127, bankB = quarters 2+3).  Output is scaled into a per-image SBUF
accumulator (bf16) and stored with 4 large DMAs (~820 KB each); the final
fp32 upcast happens on host (bf16 rounding ~0.4%% << 2e-2 tolerance).
"""
import sys
import numpy as np
from contextlib import ExitStack

sys.path.insert(0, "/root/.axon_site/_ro/trn_rl_repo")
sys.path.insert(0, "/opt/trn_rl_repo")

import ml_dtypes
import concourse.bass as bass
import concourse.bacc as bacc
import concourse.mybir as mybir
import concourse.tile as tile
from concourse.bass_utils import run_bass_kernel_spmd

F32 = mybir.dt.float32
BF16 = mybir.dt.bfloat16

N_CORES = 8
B, CIN, COUT, KS = 32, 64, 64, 3
H = W = 160
B_CORE = B // N_CORES
QH = H // 4            # output rows per quarter (40)
SQ = QH + 2            # slab rows per quarter (1 halo/pad row each side)
PW = W + 2
RPT = 3                # output rows per PSUM tile


def build_nc(n_img=B_CORE, h=H, w=W):
    qh = h // 4
    sq = qh + 2
    pw = w + 2
    nc = bacc.Bacc("TRN2", target_bir_lowering=False, debug=False, num_devices=N_CORES)
    x_in = nc.declare_dram_parameter("x", [n_img, CIN, h, w], F32, isOutput=False)
    wsgn_in = nc.declare_dram_parameter("wsgn", [128, 9 * 64], BF16, isOutput=False)
    scale_in = nc.declare_dram_parameter("scale", [128, 1], F32, isOutput=False)
    out_ext = nc.declare_dram_parameter("out", [n_img, COUT, h, w], BF16, isOutput=True)

    n_tiles = (qh + RPT - 1) // RPT  # 13 full + 1 leftover row

    with tile.TileContext(nc) as tc, ExitStack() as ctx:
        wpool = ctx.enter_context(tc.tile_pool(name="wpool", bufs=1))
        spool = ctx.enter_context(tc.tile_pool(name="spool", bufs=1))
        xpool = ctx.enter_context(tc.tile_pool(name="xpool", bufs=3))
        ppool = ctx.enter_context(tc.tile_pool(name="ppool", bufs=3, space="PSUM"))
        opool = ctx.enter_context(tc.tile_pool(name="opool", bufs=4))

        wt2 = wpool.tile([128, 9 * 64], BF16, name="wt2")
        nc.sync.dma_start(wt2[:], wsgn_in[:])
        sc2 = wpool.tile([128, 1], F32, name="sc2")
        nc.sync.dma_start(sc2[:], scale_in[:])

        # Two persistent slab pairs (manual ping-pong across images).
        # slabA: partitions 0:64 = quarter 0 (slab row s <- x row s-1),
        #        partitions 64:128 = quarter 2 (slab row s <- x row 2*qh-1+s)
        # slabB: partitions 0:64 = quarter 1 (slab row s <- x row qh-1+s),
        #        partitions 64:128 = quarter 3 (slab row s <- x row 3*qh-1+s)
        slabs = []
        for i in range(2):
            slabA = spool.tile([128, sq * pw], BF16, name=f"slabA{i}", tag=f"slabA{i}")
            slabB = spool.tile([128, sq * pw], BF16, name=f"slabB{i}", tag=f"slabB{i}")
            for slab in (slabA, slabB):
                # col pads: elements r*pw + {0, pw-1} for all slab rows
                nc.vector.memset(slab[:, 0 : (sq - 1) * pw + pw : pw], 0.0)
                nc.vector.memset(slab[:, pw - 1 : sq * pw : pw], 0.0)
            sA3 = slabA.rearrange("p (r c) -> p r c", c=pw)
            sB3 = slabB.rearrange("p (r c) -> p r c", c=pw)
            nc.vector.memset(sA3[0:64, 0, :], 0.0)        # image top pad
            nc.vector.memset(sB3[64:128, sq - 1, :], 0.0)  # image bottom pad
            slabs.append((slabA, slabB))

        # bulk chunk split of the common slab-row ranges (A: rows 1..sq-1,
        # B: rows 0..sq-2).  First chunk small so tile 0's matmuls start early.
        a_chunks = [(1, 6), (6, 24), (24, 42)]
        b_chunks = [(0, 5), (5, 23), (23, 41)]

        for img in range(n_img):
            slabA, slabB = slabs[img % 2]
            sA3 = slabA.rearrange("p (r c) -> p r c", c=pw)
            sB3 = slabB.rearrange("p (r c) -> p r c", c=pw)

            # halo rows first: slabA row 0 (bottom group) <- x row 2*qh-1,
            # slabB row sq-1 (top group) <- x row 2*qh
            xs = xpool.tile([128, w], F32, name="xs", tag="xs")
            nc.sync.dma_start(xs[64:128, :], x_in[img, :, 2 * qh - 1 : 2 * qh, :])
            nc.sync.dma_start(xs[0:64, :], x_in[img, :, 2 * qh : 2 * qh + 1, :])
            nc.scalar.sign(sA3[64:128, 0, 1 : 1 + w], xs[64:128, :])
            nc.scalar.sign(sB3[0:64, sq - 1, 1 : 1 + w], xs[0:64, :])

            for (a0, a1), (b0, b1) in zip(a_chunks, b_chunks):
                na = a1 - a0
                xa = xpool.tile([128, na * w], F32, name="xa", tag=f"xc{na}")
                xa3 = xa.rearrange("p (r c) -> p r c", c=w)
                nc.sync.dma_start(xa[0:64, :], x_in[img, :, a0 - 1 : a1 - 1, :])
                nc.sync.dma_start(
                    xa[64:128, :], x_in[img, :, 2 * qh - 1 + a0 : 2 * qh - 1 + a1, :]
                )
                nc.scalar.sign(sA3[:, a0:a1, 1 : 1 + w], xa3[:])

                nb = b1 - b0
                xb = xpool.tile([128, nb * w], F32, name="xb", tag=f"xc{nb}")
                xb3 = xb.rearrange("p (r c) -> p r c", c=w)
                nc.sync.dma_start(xb[0:64, :], x_in[img, :, qh - 1 + b0 : qh - 1 + b1, :])
                nc.sync.dma_start(
                    xb[64:128, :], x_in[img, :, 3 * qh - 1 + b0 : 3 * qh - 1 + b1, :]
                )
                nc.scalar.sign(sB3[:, b0:b1, 1 : 1 + w], xb3[:])

            obufAB = opool.tile([128, qh * w], BF16, name="obufAB", tag="obufAB")
            obufCD = opool.tile([128, qh * w], BF16, name="obufCD", tag="obufCD")

            for t in range(n_tiles):
                h0 = t * RPT
                R = min(RPT, qh - h0)
                psumA = ppool.tile([128, R * w], F32, name="psumA", tag="psumA")
                psumB = ppool.tile([128, R * w], F32, name="psumB", tag="psumB")
                for kh in range(KS):
                    for kw in range(KS):
                        pos = kh * KS + kw
                        st, sp = (pos == 0), (pos == 8)
                        wA = wt2[0:64, pos * 64 : (pos + 1) * 64]
                        wB = wt2[64:128, pos * 64 : (pos + 1) * 64]
                        nc.tensor.matmul(
                            psumA[0:64, :], wA,
                            sA3[0:64, h0 + kh : h0 + kh + R, kw : kw + w],
                            start=st, stop=sp, tile_position=(0, 0),
                        )
                        nc.tensor.matmul(
                            psumB[0:64, :], wB,
                            sA3[64:128, h0 + kh : h0 + kh + R, kw : kw + w],
                            start=st, stop=sp, tile_position=(64, 0),
                        )
                        nc.tensor.matmul(
                            psumA[64:128, :], wA,
                            sB3[0:64, h0 + kh : h0 + kh + R, kw : kw + w],
                            start=st, stop=sp, tile_position=(0, 64),
                        )
                        nc.tensor.matmul(
                            psumB[64:128, :], wB,
                            sB3[64:128, h0 + kh : h0 + kh + R, kw : kw + w],
                            start=st, stop=sp, tile_position=(64, 64),
                        )
                # scale + downcast into the per-image output accumulator (DVE)
                nc.vector.tensor_scalar_mul(
                    obufAB[:, h0 * w : (h0 + R) * w], psumA[:], sc2[:]
                )
                nc.vector.tensor_scalar_mul(
                    obufCD[:, h0 * w : (h0 + R) * w], psumB[:], sc2[:]
                )

            nc.sync.dma_start(out_ext[img, :, 0:qh, :], obufAB[0:64, :])
            nc.sync.dma_start(out_ext[img, :, qh : 2 * qh, :], obufAB[64:128, :])
            nc.sync.dma_start(out_ext[img, :, 2 * qh : 3 * qh, :], obufCD[0:64, :])
            nc.sync.dma_start(out_ext[img, :, 3 * qh : 4 * qh, :], obufCD[64:128, :])
    nc.finalize()
    return nc


_NC_CACHE = {}


def _get_nc():
    if "nc" not in _NC_CACHE:
        _NC_CACHE["nc"] = build_nc()
    return _NC_CACHE["nc"]


def _prep_weights(w):
    wc = np.clip(np.asarray(w, dtype=np.float32), -1.0, 1.0)
    scale = np.abs(wc).mean(axis=(1, 2, 3)).astype(np.float32).reshape(64, 1)
    s = np.sign(wc).astype(np.float32)  # [co, ci, kh, kw]
    wsgn = np.ascontiguousarray(
        np.transpose(s, (1, 2, 3, 0)).reshape(64, 9 * 64)
    )
    wsgn2 = np.concatenate([wsgn, wsgn], axis=0).astype(ml_dtypes.bfloat16)
    scale2 = np.concatenate([scale, scale], axis=0)
    return wsgn2, scale2


def kernel(x, w, _trace=False):
    x = np.ascontiguousarray(np.asarray(x, dtype=np.float32))
    wsgn2, scale2 = _prep_weights(w)
    nc = _get_nc()
    in_maps = [
        {"x": x[i * B_CORE : (i + 1) * B_CORE], "wsgn": wsgn2, "scale": scale2}
        for i in range(N_CORES)
    ]
    # The axon-proxied execution occasionally faults with a transient
    # NRT_EXEC_UNIT_UNRECOVERABLE; a retry on a fresh session recovers.
    last_err = None
    for attempt in range(3):
        try:
            res = run_bass_kernel_spmd(nc, in_maps, list(range(N_CORES)), trace=_trace)
            break
        except Exception as e:  # noqa: BLE001
            last_err = e
            import time as _time
            _time.sleep(3.0)
    else:
        raise last_err
    out = np.concatenate(
        [np.asarray(res.results[i]["out"]).astype(np.float32) for i in range(N_CORES)],
        axis=0,
    )
    if _trace:
        return out, res
    return out


# revision 3
# speedup vs baseline: 1.4579x; 1.3036x over previous
"""BinaryConv2d Trainium2 kernel (8-core batch-parallel, full 2x2 PE quadrant
packing).

Per image the 160 output rows split into 4 quarters of 40 rows; each quarter
is one 64x64 PE quadrant (tile_position (0,0)/(64,0)/(0,64)/(64,64)), so all
128x128 PEs are busy: quarters 0/2 stream from SBUF partitions 0:63/64:127 of
slabA, quarters 1/3 from slabB.  Each 3x3 position is 4 concurrent 64x64
matmuls accumulating into two PSUM tiles (bankA = quarters 0+1 on partitions
0:63/64:127, bankB = quarters 2+3).  Output is scaled into a per-image SBUF
accumulator (bf16) and stored with 4 large DMAs (~820 KB each); the final
fp32 upcast happens on host (bf16 rounding ~0.4%% << 2e-2 tolerance).
"""
import sys
import numpy as np
from contextlib import ExitStack

sys.path.insert(0, "/root/.axon_site/_ro/trn_rl_repo")
sys.path.insert(0, "/opt/trn_rl_repo")

import ml_dtypes
import concourse.bass as bass
import concourse.bacc as bacc
import concourse.mybir as mybir
import concourse.tile as tile
from concourse.bass_utils import run_bass_kernel_spmd

F32 = mybir.dt.float32
BF16 = mybir.dt.bfloat16

N_CORES = 8
B, CIN, COUT, KS = 32, 64, 64, 3
H = W = 160
B_CORE = B // N_CORES
QH = H // 4            # output rows per quarter (40)
SQ = QH + 2            # slab rows per quarter (1 halo/pad row each side)
PW = W + 2
RPT = 3                # output rows per PSUM tile


def build_nc(n_img=B_CORE, h=H, w=W):
    qh = h // 4
    sq = qh + 2
    pw = w + 2
    nc = bacc.Bacc("TRN2", target_bir_lowering=False, debug=False, num_devices=N_CORES)
    x_in = nc.declare_dram_parameter("x", [n_img, CIN, h, w], F32, isOutput=False)
    wsgn_in = nc.declare_dram_parameter("wsgn", [128, 9 * 64], BF16, isOutput=False)
    scale_in = nc.declare_dram_parameter("scale", [128, 1], F32, isOutput=False)
    out_ext = nc.declare_dram_parameter("out", [n_img, COUT, h, w], BF16, isOutput=True)

    n_tiles = (qh + RPT - 1) // RPT  # 13 full + 1 leftover row

    with tile.TileContext(nc) as tc, ExitStack() as ctx:
        wpool = ctx.enter_context(tc.tile_pool(name="wpool", bufs=1))
        spool = ctx.enter_context(tc.tile_pool(name="spool", bufs=1))
        xpool = ctx.enter_context(tc.tile_pool(name="xpool", bufs=3))
        ppool = ctx.enter_context(tc.tile_pool(name="ppool", bufs=3, space="PSUM"))
        opool = ctx.enter_context(tc.tile_pool(name="opool", bufs=4))

        wt2 = wpool.tile([128, 9 * 64], BF16, name="wt2")
        nc.sync.dma_start(wt2[:], wsgn_in[:])
        sc2 = wpool.tile([128, 1], F32, name="sc2")
        nc.sync.dma_start(sc2[:], scale_in[:])

        # Two persistent slab pairs (manual ping-pong across images).
        # slabA: partitions 0:64 = quarter 0 (slab row s <- x row s-1),
        #        partitions 64:128 = quarter 2 (slab row s <- x row 2*qh-1+s)
        # slabB: partitions 0:64 = quarter 1 (slab row s <- x row qh-1+s),
        #        partitions 64:128 = quarter 3 (slab row s <- x row 3*qh-1+s)
        slabs = []
        for i in range(2):
            slabA = spool.tile([128, sq * pw], BF16, name=f"slabA{i}", tag=f"slabA{i}")
            slabB = spool.tile([128, sq * pw], BF16, name=f"slabB{i}", tag=f"slabB{i}")
            for slab in (slabA, slabB):
                # col pads: elements r*pw + {0, pw-1} for all slab rows
                nc.vector.memset(slab[:, 0 : (sq - 1) * pw + pw : pw], 0.0)
                nc.vector.memset(slab[:, pw - 1 : sq * pw : pw], 0.0)
            sA3 = slabA.rearrange("p (r c) -> p r c", c=pw)
            sB3 = slabB.rearrange("p (r c) -> p r c", c=pw)
            nc.vector.memset(sA3[0:64, 0, :], 0.0)        # image top pad
            nc.vector.memset(sB3[64:128, sq - 1, :], 0.0)  # image bottom pad
            slabs.append((slabA, slabB))

        # bulk chunk split of the common slab-row ranges (A: rows 1..sq-1,
        # B: rows 0..sq-2).  First chunk small so tile 0's matmuls start early.
        a_chunks = [(1, 6), (6, 24), (24, 42)]
        b_chunks = [(0, 5), (5, 23), (23, 41)]

        for img in range(n_img):
            slabA, slabB = slabs[img % 2]
            sA3 = slabA.rearrange("p (r c) -> p r c", c=pw)
            sB3 = slabB.rearrange("p (r c) -> p r c", c=pw)

            # halo rows first: slabA row 0 (bottom group) <- x row 2*qh-1,
            # slabB row sq-1 (top group) <- x row 2*qh.  All input loads are
            # SWDGE cast-DMAs (f32 HBM -> bf16 SBUF): the SBUF AXI ports are
            # the contended resource under 8-core SPMD, so halving the
            # SBUF-side bytes halves the input's share of the port budget.
            xs = xpool.tile([128, w], BF16, name="xs", tag="xs")
            nc.gpsimd.dma_start(xs[64:128, :], x_in[img, :, 2 * qh - 1 : 2 * qh, :])
            nc.gpsimd.dma_start(xs[0:64, :], x_in[img, :, 2 * qh : 2 * qh + 1, :])
            nc.scalar.sign(sA3[64:128, 0, 1 : 1 + w], xs[64:128, :])
            nc.scalar.sign(sB3[0:64, sq - 1, 1 : 1 + w], xs[0:64, :])

            for (a0, a1), (b0, b1) in zip(a_chunks, b_chunks):
                na = a1 - a0
                xa = xpool.tile([128, na * w], BF16, name="xa", tag=f"xc{na}")
                xa3 = xa.rearrange("p (r c) -> p r c", c=w)
                nc.gpsimd.dma_start(xa[0:64, :], x_in[img, :, a0 - 1 : a1 - 1, :])
                nc.gpsimd.dma_start(
                    xa[64:128, :], x_in[img, :, 2 * qh - 1 + a0 : 2 * qh - 1 + a1, :]
                )
                nc.scalar.sign(sA3[:, a0:a1, 1 : 1 + w], xa3[:])

                nb = b1 - b0
                xb = xpool.tile([128, nb * w], BF16, name="xb", tag=f"xc{nb}")
                xb3 = xb.rearrange("p (r c) -> p r c", c=w)
                nc.gpsimd.dma_start(xb[0:64, :], x_in[img, :, qh - 1 + b0 : qh - 1 + b1, :])
                nc.gpsimd.dma_start(
                    xb[64:128, :], x_in[img, :, 3 * qh - 1 + b0 : 3 * qh - 1 + b1, :]
                )
                nc.scalar.sign(sB3[:, b0:b1, 1 : 1 + w], xb3[:])

            obufAB = opool.tile([128, qh * w], BF16, name="obufAB", tag="obufAB")
            obufCD = opool.tile([128, qh * w], BF16, name="obufCD", tag="obufCD")

            for t in range(n_tiles):
                h0 = t * RPT
                R = min(RPT, qh - h0)
                psumA = ppool.tile([128, R * w], F32, name="psumA", tag="psumA")
                psumB = ppool.tile([128, R * w], F32, name="psumB", tag="psumB")
                for kh in range(KS):
                    for kw in range(KS):
                        pos = kh * KS + kw
                        st, sp = (pos == 0), (pos == 8)
                        wA = wt2[0:64, pos * 64 : (pos + 1) * 64]
                        wB = wt2[64:128, pos * 64 : (pos + 1) * 64]
                        nc.tensor.matmul(
                            psumA[0:64, :], wA,
                            sA3[0:64, h0 + kh : h0 + kh + R, kw : kw + w],
                            start=st, stop=sp, tile_position=(0, 0),
                        )
                        nc.tensor.matmul(
                            psumB[0:64, :], wB,
                            sA3[64:128, h0 + kh : h0 + kh + R, kw : kw + w],
                            start=st, stop=sp, tile_position=(64, 0),
                        )
                        nc.tensor.matmul(
                            psumA[64:128, :], wA,
                            sB3[0:64, h0 + kh : h0 + kh + R, kw : kw + w],
                            start=st, stop=sp, tile_position=(0, 64),
                        )
                        nc.tensor.matmul(
                            psumB[64:128, :], wB,
                            sB3[64:128, h0 + kh : h0 + kh + R, kw : kw + w],
                            start=st, stop=sp, tile_position=(64, 64),
                        )
                # scale + downcast into the per-image output accumulator (DVE)
                nc.vector.tensor_scalar_mul(
                    obufAB[:, h0 * w : (h0 + R) * w], psumA[:], sc2[:]
                )
                nc.vector.tensor_scalar_mul(
                    obufCD[:, h0 * w : (h0 + R) * w], psumB[:], sc2[:]
                )

            nc.sync.dma_start(out_ext[img, :, 0:qh, :], obufAB[0:64, :])
            nc.sync.dma_start(out_ext[img, :, qh : 2 * qh, :], obufAB[64:128, :])
            nc.sync.dma_start(out_ext[img, :, 2 * qh : 3 * qh, :], obufCD[0:64, :])
            nc.sync.dma_start(out_ext[img, :, 3 * qh : 4 * qh, :], obufCD[64:128, :])
    nc.finalize()
    return nc


_NC_CACHE = {}


def _get_nc():
    if "nc" not in _NC_CACHE:
        _NC_CACHE["nc"] = build_nc()
    return _NC_CACHE["nc"]


def _prep_weights(w):
    wc = np.clip(np.asarray(w, dtype=np.float32), -1.0, 1.0)
    scale = np.abs(wc).mean(axis=(1, 2, 3)).astype(np.float32).reshape(64, 1)
    s = np.sign(wc).astype(np.float32)  # [co, ci, kh, kw]
    wsgn = np.ascontiguousarray(
        np.transpose(s, (1, 2, 3, 0)).reshape(64, 9 * 64)
    )
    wsgn2 = np.concatenate([wsgn, wsgn], axis=0).astype(ml_dtypes.bfloat16)
    scale2 = np.concatenate([scale, scale], axis=0)
    return wsgn2, scale2


def kernel(x, w, _trace=False):
    x = np.ascontiguousarray(np.asarray(x, dtype=np.float32))
    wsgn2, scale2 = _prep_weights(w)
    nc = _get_nc()
    in_maps = [
        {"x": x[i * B_CORE : (i + 1) * B_CORE], "wsgn": wsgn2, "scale": scale2}
        for i in range(N_CORES)
    ]
    # The axon-proxied execution occasionally faults with a transient
    # NRT_EXEC_UNIT_UNRECOVERABLE; a retry on a fresh session recovers.
    last_err = None
    for attempt in range(3):
        try:
            res = run_bass_kernel_spmd(nc, in_maps, list(range(N_CORES)), trace=_trace)
            break
        except Exception as e:  # noqa: BLE001
            last_err = e
            import time as _time
            _time.sleep(3.0)
    else:
        raise last_err
    out = np.concatenate(
        [np.asarray(res.results[i]["out"]).astype(np.float32) for i in range(N_CORES)],
        axis=0,
    )
    if _trace:
        return out, res
    return out
